# revision 2
# baseline (speedup 1.0000x reference)
"""AFNO (Adaptive Fourier Neural Operator) Trainium2 kernel.

Data-parallel over batch: 32 batches -> 8 cores x 4 batches.
Per core, per (batch, c-quarter=block) the pipeline is 6 matmul phases with
DMA layout rotations between them (all compute bf16, fp32 PSUM):

  x (w | h,c)            --Wfft-->   YW ((r,w') | h,c)      [rfft over w]
  rot1 (DMA)             -->         YH ((r,h)  | w',c)
  Hfft (2 stationaries)  -->         XF (h' | w',c,r)       [fft over h]
  rot2 (DMA)             -->         XM ((2c+r) | h',w') x3  [c to partitions]
  M1 (complex-packed)    -->         HM  (relu(.+b1))
  M2                     -->         OM  (softshrink(.+b2))
  rot3 (DMA)             -->         OC (h' | w',c,r)
  iH (2-pass accum)      -->         VH ((r,h) | w',c)
  rot4 (DMA)             -->         VW ((r,w') | h,c)
  iW                     -->         OUT (w | h,c) -> DRAM

DFT matrices and complex/block-packed weights are precomputed host-side and
passed as extra inputs.
"""
import numpy as np
import ml_dtypes
from contextlib import ExitStack

import concourse.bass as bass
import concourse.mybir as mybir
import concourse.tile as tile
from concourse import bacc
from concourse.bass_utils import run_bass_kernel_spmd
from concourse.masks import make_identity

H = 56
W = 56
WF = 29
C = 768
NB = 4
BS = 192
LAM = 0.01
NCORES = 8
B_FULL = 32
BPC = B_FULL // NCORES  # 4

F32 = mybir.dt.float32
BF16 = mybir.dt.bfloat16
AF = mybir.ActivationFunctionType
ALU = mybir.AluOpType

BF = ml_dtypes.bfloat16


def make_consts(w1, b1, w2, b2):
    """Pack DFT matrices and mixing weights/biases host-side (numpy)."""
    w = np.arange(W)
    wp = np.arange(WF)
    ang = 2 * np.pi * np.outer(wp, w) / W
    Cw = np.cos(ang) / np.sqrt(W)
    Sw = np.sin(ang) / np.sqrt(W)
    h = np.arange(H)
    angh = 2 * np.pi * np.outer(h, h) / H
    Ch = np.cos(angh) / np.sqrt(H)  # [h', h] (symmetric)
    Sh = np.sin(angh) / np.sqrt(H)
    Chi, Shi = Ch, Sh  # cos/sin(2pi h h'/H)/sqrt(H); symmetric matrices
    alpha = np.full(WF, 2.0)
    alpha[0] = 1.0
    alpha[WF - 1] = 1.0
    A = (alpha[None, :] * np.cos(2 * np.pi * np.outer(w, wp) / W)) / np.sqrt(W)
    Bm = (-alpha[None, :] * np.sin(2 * np.pi * np.outer(w, wp) / W)) / np.sqrt(W)
    Bm[:, 0] = 0.0
    Bm[:, WF - 1] = 0.0

    # Wfft stationary: (56w, 58): cols [Yr = Cw x | Yi = -Sw x]
    fw = np.zeros((W, 58), np.float32)
    fw[:, :WF] = Cw.T
    fw[:, WF:] = -Sw.T

    # Hfft stationaries, moving rows = [Yr(h) ; Yi(h)] (112):
    fhr = np.zeros((112, H), np.float32)  # -> Xr = Ch Yr + Sh Yi
    fhr[:H] = Ch.T
    fhr[H:] = Sh.T
    fhi = np.zeros((112, H), np.float32)  # -> Xi = Ch Yi - Sh Yr
    fhi[:H] = -Sh.T
    fhi[H:] = Ch.T

    # iH stationaries: moving = Or (pass r) / Oi (pass i), psum = [Vr ; Vi]
    ghr = np.zeros((H, 112), np.float32)
    ghr[:, :H] = Chi.T  # Vr += Chi @ Or  -> ghr[h', m] = Chi[m, h']
    ghr[:, H:] = Shi.T  # Vi += Shi @ Or
    ghi = np.zeros((H, 112), np.float32)
    ghi[:, :H] = -Shi.T  # Vr -= Shi @ Oi
    ghi[:, H:] = Chi.T  # Vi += Chi @ Oi

    # iW stationary: moving rows = [Vr(w') ; Vi(w')] (58) -> out[w]
    gw = np.zeros((58, W), np.float32)
    gw[:WF] = A.T
    gw[WF:] = Bm.T

    # Mixing weights, complex-interleaved on both sides.
    # rows 2j+r over d=64kc+j; cols 2i+rp over k=64mc+i
    def pack_mix(wl):
        wr, wi = wl[0], wl[1]  # (NB, 192, 192)
        m = np.zeros((NB, 3, 3, 128, 128), np.float32)
        for blk in range(NB):
            for kc in range(3):
                ds = slice(64 * kc, 64 * kc + 64)
                for mc in range(3):
                    ks = slice(64 * mc, 64 * mc + 64)
                    blkr = wr[blk][ds, ks]  # (64, 64)
                    blki = wi[blk][ds, ks]
                    t = m[blk, kc, mc]
                    t[0::2, 0::2] = blkr   # r=0 -> Zr
                    t[1::2, 0::2] = -blki  # r=1 -> Zr
                    t[0::2, 1::2] = blki   # r=0 -> Zi
                    t[1::2, 1::2] = blkr   # r=1 -> Zi
        return m

    m1 = pack_mix(w1)
    m2 = pack_mix(w2)

    def pack_bias(bl, scale=1.0, off=0.0):
        # (NB*3, 128): [2i+rp] = scale*bl[rp][blk][64mc+i] + off
        out = np.zeros((NB * 3, 128), np.float32)
        for blk in range(NB):
            for mc in range(3):
                ks = slice(64 * mc, 64 * mc + 64)
                out[blk * 3 + mc, 0::2] = scale * bl[0][blk][ks] + off
                out[blk * 3 + mc, 1::2] = scale * bl[1][blk][ks] + off
        return out

    b1p = pack_bias(b1)
    b2e1 = pack_bias(b2, 1.0, -LAM)    # relu(v + b2 - lam)
    b2e2 = pack_bias(b2, -1.0, -LAM)   # relu(-v - b2 - lam)

    cb = lambda a: np.ascontiguousarray(a.astype(BF))
    cf = lambda a: np.ascontiguousarray(a.astype(np.float32))
    return {
        "fw": cb(fw), "fhr": cb(fhr), "fhi": cb(fhi),
        "ghr": cb(ghr), "ghi": cb(ghi), "gw": cb(gw),
        "m1": cb(m1), "m2": cb(m2),
        "b1p": cf(b1p), "b2e1": cf(b2e1), "b2e2": cf(b2e2),
    }


def build_nc(n_b=BPC, n_cq=NB):
    nc = bacc.Bacc(None, target_bir_lowering=False, debug=False)

    x_ext = nc.declare_dram_parameter("x", [n_b, H, W, C], F32, isOutput=False)
    out_ext = nc.declare_dram_parameter("out", [n_b, H, W, C], F32, isOutput=True)
    fw_e = nc.declare_dram_parameter("fw", [W, 58], BF16, isOutput=False)
    fhr_e = nc.declare_dram_parameter("fhr", [112, H], BF16, isOutput=False)
    fhi_e = nc.declare_dram_parameter("fhi", [112, H], BF16, isOutput=False)
    ghr_e = nc.declare_dram_parameter("ghr", [H, 112], BF16, isOutput=False)
    ghi_e = nc.declare_dram_parameter("ghi", [H, 112], BF16, isOutput=False)
    gw_e = nc.declare_dram_parameter("gw", [58, W], BF16, isOutput=False)
    m1_e = nc.declare_dram_parameter("m1", [NB, 3, 3, 128, 128], BF16, isOutput=False)
    m2_e = nc.declare_dram_parameter("m2", [NB, 3, 3, 128, 128], BF16, isOutput=False)
    b1p_e = nc.declare_dram_parameter("b1p", [NB * 3, 128], F32, isOutput=False)
    b2e1_e = nc.declare_dram_parameter("b2e1", [NB * 3, 128], F32, isOutput=False)
    b2e2_e = nc.declare_dram_parameter("b2e2", [NB * 3, 128], F32, isOutput=False)

    ev = [0]  # eviction engine round-robin counter

    with tile.TileContext(nc) as tc, ExitStack() as ctx:
        consts = ctx.enter_context(tc.tile_pool(name="consts", bufs=1))
        big = ctx.enter_context(tc.tile_pool(name="big", bufs=1))
        mid = ctx.enter_context(tc.tile_pool(name="mid", bufs=1))
        xmp = ctx.enter_context(tc.tile_pool(name="xmp", bufs=3))
        hmp = ctx.enter_context(tc.tile_pool(name="hmp", bufs=3))
        omp = ctx.enter_context(tc.tile_pool(name="omp", bufs=3))
        tmp = ctx.enter_context(tc.tile_pool(name="tmp", bufs=2))
        ps = ctx.enter_context(tc.tile_pool(name="ps", bufs=4, space="PSUM"))
        dram = ctx.enter_context(tc.tile_pool(name="dram", bufs=2, space="DRAM"))

        # ---- load constants ----
        fw_t = consts.tile([W, 58], BF16, tag="c1")
        nc.sync.dma_start(out=fw_t, in_=fw_e[:, :])
        fhr_t = consts.tile([112, H], BF16, tag="c2")
        nc.sync.dma_start(out=fhr_t, in_=fhr_e[:, :])
        fhi_t = consts.tile([112, H], BF16, tag="c3")
        nc.sync.dma_start(out=fhi_t, in_=fhi_e[:, :])
        ghr_t = consts.tile([H, 112], BF16, tag="c4")
        nc.sync.dma_start(out=ghr_t, in_=ghr_e[:, :])
        ghi_t = consts.tile([H, 112], BF16, tag="c5")
        nc.sync.dma_start(out=ghi_t, in_=ghi_e[:, :])
        gw_t = consts.tile([58, W], BF16, tag="c6")
        nc.sync.dma_start(out=gw_t, in_=gw_e[:, :])
        m1_t = consts.tile([128, NB, 3, 3, 128], BF16, tag="c7")
        nc.sync.dma_start(out=m1_t, in_=m1_e[:, :, :, :, :].transpose((3, 0, 1, 2, 4)))
        m2_t = consts.tile([128, NB, 3, 3, 128], BF16, tag="c8")
        nc.sync.dma_start(out=m2_t, in_=m2_e[:, :, :, :, :].transpose((3, 0, 1, 2, 4)))
        b1p_t = consts.tile([128, NB * 3], F32, tag="c9")
        nc.sync.dma_start(out=b1p_t, in_=b1p_e[:, :].transpose((1, 0)))
        b2e1_t = consts.tile([128, NB * 3], F32, tag="ca")
        nc.sync.dma_start(out=b2e1_t, in_=b2e1_e[:, :].transpose((1, 0)))
        b2e2_t = consts.tile([128, NB * 3], F32, tag="cb")
        nc.sync.dma_start(out=b2e2_t, in_=b2e2_e[:, :].transpose((1, 0)))
        ident = consts.tile([128, 128], BF16, tag="cid")
        make_identity(nc, ident[:, :])

        def evict(dst, src):
            # round-robin: 2/3 DVE, 1/3 ACT
            if ev[0] % 3 == 2:
                nc.scalar.activation(dst, src, AF.Copy)
            else:
                nc.vector.tensor_copy(dst, src)
            ev[0] += 1

        NHC = H * BS            # 10752 = 21*512
        NWC = WF * BS           # 5568 = 10*512 + 448
        NPT = H * WF            # 1624 = 4*406

        for b in range(n_b):
            for cq in range(n_cq):
                cs = slice(cq * BS, (cq + 1) * BS)

                # ---- S1: load x[b] c-slice as (w | h, c), cast f32->bf16
                xw = big.tile([W, H, BS], BF16, tag="xw")
                nc.gpsimd.dma_start(
                    out=xw, in_=x_ext[b, :, :, cs].transpose((1, 0, 2)))
                xw_f = xw[:, :, :].rearrange("w h c -> w (h c)")

                # ---- S2: Wfft -> YW ((r,w') | h, c)
                yw = big.tile([58, H, BS], BF16, tag="yw")
                yw_f = yw[:, :, :].rearrange("p h c -> p (h c)")
                for j in range(21):
                    sl = slice(512 * j, 512 * (j + 1))
                    pw = ps.tile([58, 512], F32, tag="ps")
                    nc.tensor.matmul(pw[:, :], fw_t[:, :], xw_f[:, sl],
                                     start=True, stop=True)
                    evict(yw_f[:, sl], pw[:, :])

                # ---- S3: rot1 -> YH ((r,h) | w', c), via DRAM bounce
                s1 = dram.tile([58, H, BS], BF16, tag="s1")
                nc.sync.dma_start(out=s1[:, :, :], in_=yw[:, :, :])
                yh = mid.tile([112, WF, BS], BF16, tag="yh")
                for r in range(2):
                    nc.sync.dma_start(
                        out=yh[56 * r:56 * r + 56, :, :],
                        in_=s1[29 * r:29 * r + 29, :, :].transpose((1, 0, 2)))
                yh_f = yh[:, :, :].rearrange("p w c -> p (w c)")

                # ---- S4: Hfft -> XF (h' | w', c, r)
                xf = big.tile([H, WF, BS, 2], BF16, tag="xf")
                xf_r = xf[:, :, :, 0].rearrange("q w c -> q (w c)")
                xf_i = xf[:, :, :, 1].rearrange("q w c -> q (w c)")
                for j in range(11):
                    lo = 512 * j
                    hi = min(512 * (j + 1), NWC)
                    n = hi - lo
                    sl = slice(lo, hi)
                    pxr = ps.tile([H, 512], F32, tag="ps")
                    pxi = ps.tile([H, 512], F32, tag="ps")
                    nc.tensor.matmul(pxr[:, :n], fhr_t[:, :], yh_f[:, sl],
                                     start=True, stop=True)
                    nc.tensor.matmul(pxi[:, :n], fhi_t[:, :], yh_f[:, sl],
                                     start=True, stop=True)
                    evict(xf_r[:, sl], pxr[:, :n])
                    evict(xf_i[:, sl], pxi[:, :n])

                # ---- S5: rot2 -> XM[kc] ((2c+r) | h', w') via PE transpose
                xm = []
                for kc in range(3):
                    t = xmp.tile([128, H, WF], BF16, tag="xm", name=f"xm{b}_{cq}_{kc}")
                    for wp in range(WF):
                        pt = ps.tile([128, H], BF16, tag="pst")
                        nc.tensor.transpose(
                            pt[:, :],
                            xf[:, wp, 64 * kc:64 * kc + 64, :].rearrange(
                                "q c r -> q (c r)"),
                            ident[:H, :H])
                        evict(t[:, :, wp], pt[:, :])
                    xm.append(t)
                xm_f = [t[:, :, :].rearrange("p h w -> p (h w)") for t in xm]

                # ---- S6: M1 (relu(. + b1)) -> HM
                hm = [hmp.tile([128, H, WF], BF16, tag="hm", name=f"hm{b}_{cq}_{i}") for i in range(3)]
                hm_f = [t[:, :, :].rearrange("p h w -> p (h w)") for t in hm]
                for mc in range(3):
                    bidx = cq * 3 + mc
                    for j in range(4):
                        sl = slice(406 * j, 406 * (j + 1))
                        pm = ps.tile([128, 406], F32, tag="ps")
                        for kc in range(3):
                            nc.tensor.matmul(
                                pm[:, :], m1_t[:, cq, kc, mc, :], xm_f[kc][:, sl],
                                start=(kc == 0), stop=(kc == 2))
                        nc.vector.tensor_scalar(
                            hm_f[mc][:, sl], pm[:, :],
                            b1p_t[:, bidx:bidx + 1], 0.0, ALU.add, ALU.max)

                # ---- S7: M2 (softshrink(. + b2)) -> OM
                om = [omp.tile([128, H, WF], BF16, tag="om", name=f"om{b}_{cq}_{i}") for i in range(3)]
                om_f = [t[:, :, :].rearrange("p h w -> p (h w)") for t in om]
                for mc in range(3):
                    bidx = cq * 3 + mc
                    for j in range(4):
                        sl = slice(406 * j, 406 * (j + 1))
                        pm = ps.tile([128, 406], F32, tag="ps")
                        for kc in range(3):
                            nc.tensor.matmul(
                                pm[:, :], m2_t[:, cq, kc, mc, :], hm_f[kc][:, sl],
                                start=(kc == 0), stop=(kc == 2))
                        # e1 = relu(v + b2 - lam); e2 = relu(-v - b2 - lam)
                        e2 = tmp.tile([128, 406], BF16, tag="e2")
                        nc.scalar.activation(
                            om_f[mc][:, sl], pm[:, :], AF.Relu,
                            bias=b2e1_t[:, bidx:bidx + 1], scale=1.0)
                        nc.scalar.activation(
                            e2[:, :], pm[:, :], AF.Relu,
                            bias=b2e2_t[:, bidx:bidx + 1], scale=-1.0)
                        nc.vector.tensor_tensor(
                            om_f[mc][:, sl], om_f[mc][:, sl], e2[:, :],
                            ALU.subtract)

                # ---- S8: rot3 -> OC (h' | w', c, r) via PE transpose
                oc = big.tile([H, WF, BS, 2], BF16, tag="xf")
                for mc in range(3):
                    for wp in range(WF):
                        pt2 = ps.tile([H, 128], BF16, tag="pst")
                        nc.tensor.transpose(
                            pt2[:, :], om[mc][:, :, wp], ident[:, :])
                        evict(oc[:, wp, 64 * mc:64 * mc + 64, :].rearrange(
                            "q c r -> q (c r)"), pt2[:, :])
                oc_r = oc[:, :, :, 0].rearrange("q w c -> q (w c)")
                oc_i = oc[:, :, :, 1].rearrange("q w c -> q (w c)")

                # ---- S9: iH -> VH ((r,h) | w', c)
                vh = mid.tile([112, WF, BS], BF16, tag="vh")
                vh_f = vh[:, :, :].rearrange("p w c -> p (w c)")
                for j in range(11):
                    lo = 512 * j
                    hi = min(512 * (j + 1), NWC)
                    n = hi - lo
                    sl = slice(lo, hi)
                    pv = ps.tile([112, 512], F32, tag="ps")
                    nc.tensor.matmul(pv[:, :n], ghr_t[:, :], oc_r[:, sl],
                                     start=True, stop=False)
                    nc.tensor.matmul(pv[:, :n], ghi_t[:, :], oc_i[:, sl],
                                     start=False, stop=True)
                    evict(vh_f[:, sl], pv[:, :n])

                # ---- S10: rot4 -> VW ((r,w') | h, c), via DRAM bounce
                s4 = dram.tile([112, WF, BS], BF16, tag="s4")
                nc.sync.dma_start(out=s4[:, :, :], in_=vh[:, :, :])
                vw = big.tile([58, H, BS], BF16, tag="yw2")
                for r in range(2):
                    nc.sync.dma_start(
                        out=vw[29 * r:29 * r + 29, :, :],
                        in_=s4[56 * r:56 * r + 56, :, :].transpose((1, 0, 2)))
                vw_f = vw[:, :, :].rearrange("p h c -> p (h c)")

                # ---- S11: iW -> OUT (w | h, c)
                outt = big.tile([W, H, BS], BF16, tag="xw2")
                outt_f = outt[:, :, :].rearrange("w h c -> w (h c)")
                for j in range(21):
                    sl = slice(512 * j, 512 * (j + 1))
                    po = ps.tile([W, 512], F32, tag="ps")
                    nc.tensor.matmul(po[:, :], gw_t[:, :], vw_f[:, sl],
                                     start=True, stop=True)
                    evict(outt_f[:, sl], po[:, :])

                # ---- S12: store (cast bf16->f32)
                nc.gpsimd.dma_start(
                    out=out_ext[b, :, :, cs].transpose((1, 0, 2)), in_=outt)

    nc.compile()
    return nc


_NC_CACHE = {}


def _get_nc(n_b, n_cq):
    key = (n_b, n_cq)
    if key not in _NC_CACHE:
        _NC_CACHE[key] = build_nc(n_b, n_cq)
    return _NC_CACHE[key]


def build_in_maps(inputs):
    x = np.ascontiguousarray(np.asarray(inputs["x"], dtype=np.float32))
    consts = make_consts(np.asarray(inputs["w1"]), np.asarray(inputs["b1"]),
                         np.asarray(inputs["w2"]), np.asarray(inputs["b2"]))
    in_maps = []
    for core in range(NCORES):
        shard = np.ascontiguousarray(
            x[core * BPC:(core + 1) * BPC].reshape(BPC, H, W, C))
        m = {"x": shard}
        m.update(consts)
        in_maps.append(m)
    return in_maps


def kernel(x, w1, b1, w2, b2):
    in_maps = build_in_maps(dict(x=x, w1=w1, b1=b1, w2=w2, b2=b2))
    nc = _get_nc(BPC, NB)
    res = run_bass_kernel_spmd(nc, in_maps, core_ids=list(range(NCORES)))
    out = np.concatenate(
        [res.results[i]["out"].reshape(BPC, H * W, C) for i in range(NCORES)],
        axis=0)
    return out.astype(np.float32)



# revision 10
# speedup vs baseline: 1.3843x; 1.3843x over previous
"""AFNO (Adaptive Fourier Neural Operator) Trainium2 kernel.

Data-parallel over batch: 32 batches -> 8 cores x 4 batches.
Per core, per (batch, c-quarter=block) the pipeline is 6 matmul phases with
DMA layout rotations between them (all compute bf16, fp32 PSUM):

  x (w | h,c)            --Wfft-->   YW ((fh,r,w') | h/2,c)   [rfft over w]
  rot1 (DMA)             -->         YH ((r,h)  | w',c)
  Hfft (2 stationaries)  -->         XF (h' | w',c,r)       [fft over h]
  rot2 (PE transpose)    -->         XM ((2c+r) | h',w') x3  [c to partitions]
  M1 (complex-packed)    -->         HM  (relu(.+b1))
  M2                     -->         OM  (softshrink(.+b2))
  rot3 (PE transpose)    -->         OC ((wpar,h') | w'/2,c,r)
  iH (2-pass accum)      -->         VH ((r,h) | wpar,w'/2,c)
  rot4 (DMA)             -->         VW ((r,w') | h,c)
  iW                     -->         OUT ((fh,w) | h/2,c) -> DRAM

Throughput notes vs the naive version: PSUM evictions are partition-packed
(two 56/58-row matmul results stacked into one 112/116-row PSUM tile and
evicted once - DVE/ACT cost scales only with free-dim elements), softshrink
is computed as u - clamp(u, -lam, lam) with the subtract on the otherwise
idle GPSIMD engine, and evictions are weight-balanced across DVE and ACT.
"""
import numpy as np
import ml_dtypes
from contextlib import ExitStack

import concourse.bass as bass
import concourse.mybir as mybir
import concourse.tile as tile
from concourse import bacc
from concourse.bass_utils import run_bass_kernel_spmd
from concourse.masks import make_identity

H = 56
W = 56
WF = 29
C = 768
NB = 4
BS = 192
LAM = 0.01
NCORES = 8
B_FULL = 32
BPC = B_FULL // NCORES  # 4

F32 = mybir.dt.float32
BF16 = mybir.dt.bfloat16
AF = mybir.ActivationFunctionType
ALU = mybir.AluOpType

BF = ml_dtypes.bfloat16


def make_consts(w1, b1, w2, b2):
    """Pack DFT matrices and mixing weights/biases host-side (numpy)."""
    w = np.arange(W)
    wp = np.arange(WF)
    ang = 2 * np.pi * np.outer(wp, w) / W
    Cw = np.cos(ang) / np.sqrt(W)
    Sw = np.sin(ang) / np.sqrt(W)
    h = np.arange(H)
    angh = 2 * np.pi * np.outer(h, h) / H
    Ch = np.cos(angh) / np.sqrt(H)  # [h', h] (symmetric)
    Sh = np.sin(angh) / np.sqrt(H)
    Chi, Shi = Ch, Sh  # cos/sin(2pi h h'/H)/sqrt(H); symmetric matrices
    alpha = np.full(WF, 2.0)
    alpha[0] = 1.0
    alpha[WF - 1] = 1.0
    A = (alpha[None, :] * np.cos(2 * np.pi * np.outer(w, wp) / W)) / np.sqrt(W)
    Bm = (-alpha[None, :] * np.sin(2 * np.pi * np.outer(w, wp) / W)) / np.sqrt(W)
    Bm[:, 0] = 0.0
    Bm[:, WF - 1] = 0.0

    # Wfft stationary: (56w, 58): cols [Yr = Cw x | Yi = -Sw x]
    fw = np.zeros((W, 58), np.float32)
    fw[:, :WF] = Cw.T
    fw[:, WF:] = -Sw.T

    # Hfft stationaries, moving rows = [Yr(h) ; Yi(h)] (112):
    fhr = np.zeros((112, H), np.float32)  # -> Xr = Ch Yr + Sh Yi
    fhr[:H] = Ch.T
    fhr[H:] = Sh.T
    fhi = np.zeros((112, H), np.float32)  # -> Xi = Ch Yi - Sh Yr
    fhi[:H] = -Sh.T
    fhi[H:] = Ch.T

    # iH stationaries: moving = Or (pass r) / Oi (pass i), psum = [Vr ; Vi]
    # rows duplicated at partition offsets 0 and 64 so both w'-parity
    # slices of OC (base partitions 0 / 64) can use an aligned stationary
    ghr = np.zeros((120, 112), np.float32)
    ghr[:H, :H] = Chi.T  # Vr += Chi @ Or  -> ghr[h', m] = Chi[m, h']
    ghr[:H, H:] = Shi.T  # Vi += Shi @ Or
    ghr[64:64 + H] = ghr[:H]
    ghi = np.zeros((120, 112), np.float32)
    ghi[:H, :H] = -Shi.T  # Vr -= Shi @ Oi
    ghi[:H, H:] = Chi.T  # Vi += Chi @ Oi
    ghi[64:64 + H] = ghi[:H]

    # iW stationary: moving rows = [Vr(w') ; Vi(w')] (58) -> out[w],
    # with w' rows permuted parity-major (evens then odds) to match the
    # rot4 read layout: row (r, par, wh) <- original (r, w'=2*wh+par)
    gw0 = np.zeros((58, W), np.float32)
    gw0[:WF] = A.T
    gw0[WF:] = Bm.T
    perm = []
    for r in range(2):
        for par in range(2):
            for wh in range(15 if par == 0 else 14):
                perm.append(29 * r + 2 * wh + par)
    gw = gw0[np.array(perm)]

    # Mixing weights, complex-interleaved on both sides.
    # rows 2j+r over d=64kc+j; cols 2i+rp over k=64mc+i
    def pack_mix(wl):
        wr, wi = wl[0], wl[1]  # (NB, 192, 192)
        m = np.zeros((NB, 3, 3, 128, 128), np.float32)
        for blk in range(NB):
            for kc in range(3):
                ds = slice(64 * kc, 64 * kc + 64)
                for mc in range(3):
                    ks = slice(64 * mc, 64 * mc + 64)
                    blkr = wr[blk][ds, ks]  # (64, 64)
                    blki = wi[blk][ds, ks]
                    t = m[blk, kc, mc]
                    t[0::2, 0::2] = blkr   # r=0 -> Zr
                    t[1::2, 0::2] = -blki  # r=1 -> Zr
                    t[0::2, 1::2] = blki   # r=0 -> Zi
                    t[1::2, 1::2] = blkr   # r=1 -> Zi
        return m

    m1 = pack_mix(w1)
    m2 = pack_mix(w2)

    def pack_bias(bl, scale=1.0, off=0.0):
        # (NB*3, 128): [2i+rp] = scale*bl[rp][blk][64mc+i] + off
        out = np.zeros((NB * 3, 128), np.float32)
        for blk in range(NB):
            for mc in range(3):
                ks = slice(64 * mc, 64 * mc + 64)
                out[blk * 3 + mc, 0::2] = scale * bl[0][blk][ks] + off
                out[blk * 3 + mc, 1::2] = scale * bl[1][blk][ks] + off
        return out

    b1p = pack_bias(b1)
    b2e1 = pack_bias(b2, 1.0, -LAM)    # relu(v + b2 - lam)
    b2e2 = pack_bias(b2, -1.0, -LAM)   # relu(-v - b2 - lam)

    cb = lambda a: np.ascontiguousarray(a.astype(BF))
    cf = lambda a: np.ascontiguousarray(a.astype(np.float32))
    return {
        "fw": cb(fw), "fhr": cb(fhr), "fhi": cb(fhi),
        "ghr": cb(ghr), "ghi": cb(ghi), "gw": cb(gw),
        "m1": cb(m1), "m2": cb(m2),
        "b1p": cf(b1p), "b2e1": cf(b2e1), "b2e2": cf(b2e2),
    }


def build_nc(n_b=BPC, n_cq=NB):
    nc = bacc.Bacc(None, target_bir_lowering=False, debug=False)

    x_ext = nc.declare_dram_parameter("x", [n_b, H, W, C], F32, isOutput=False)
    out_ext = nc.declare_dram_parameter("out", [n_b, H, W, C], F32, isOutput=True)
    fw_e = nc.declare_dram_parameter("fw", [W, 58], BF16, isOutput=False)
    fhr_e = nc.declare_dram_parameter("fhr", [112, H], BF16, isOutput=False)
    fhi_e = nc.declare_dram_parameter("fhi", [112, H], BF16, isOutput=False)
    ghr_e = nc.declare_dram_parameter("ghr", [120, 112], BF16, isOutput=False)
    ghi_e = nc.declare_dram_parameter("ghi", [120, 112], BF16, isOutput=False)
    gw_e = nc.declare_dram_parameter("gw", [58, W], BF16, isOutput=False)
    m1_e = nc.declare_dram_parameter("m1", [NB, 3, 3, 128, 128], BF16, isOutput=False)
    m2_e = nc.declare_dram_parameter("m2", [NB, 3, 3, 128, 128], BF16, isOutput=False)
    b1p_e = nc.declare_dram_parameter("b1p", [NB * 3, 128], F32, isOutput=False)
    b2e1_e = nc.declare_dram_parameter("b2e1", [NB * 3, 128], F32, isOutput=False)
    b2e2_e = nc.declare_dram_parameter("b2e2", [NB * 3, 128], F32, isOutput=False)

    ev = [0]  # eviction engine round-robin counter

    with tile.TileContext(nc) as tc, ExitStack() as ctx:
        consts = ctx.enter_context(tc.tile_pool(name="consts", bufs=1))
        big = ctx.enter_context(tc.tile_pool(name="big", bufs=1))
        mid = ctx.enter_context(tc.tile_pool(name="mid", bufs=1))
        xmp = ctx.enter_context(tc.tile_pool(name="xmp", bufs=3))
        hmp = ctx.enter_context(tc.tile_pool(name="hmp", bufs=3))
        omp = ctx.enter_context(tc.tile_pool(name="omp", bufs=3))
        tmp = ctx.enter_context(tc.tile_pool(name="tmp", bufs=3))
        ps = ctx.enter_context(tc.tile_pool(name="ps", bufs=4, space="PSUM"))
        pst = ctx.enter_context(tc.tile_pool(name="pst", bufs=4, space="PSUM"))
        dram = ctx.enter_context(tc.tile_pool(name="dram", bufs=2, space="DRAM"))

        # ---- load constants ----
        fw_t = consts.tile([W, 58], BF16, tag="c1")
        nc.sync.dma_start(out=fw_t, in_=fw_e[:, :])
        fhr_t = consts.tile([112, H], BF16, tag="c2")
        nc.sync.dma_start(out=fhr_t, in_=fhr_e[:, :])
        fhi_t = consts.tile([112, H], BF16, tag="c3")
        nc.sync.dma_start(out=fhi_t, in_=fhi_e[:, :])
        ghr_t = consts.tile([120, 112], BF16, tag="c4")
        nc.sync.dma_start(out=ghr_t, in_=ghr_e[:, :])
        ghi_t = consts.tile([120, 112], BF16, tag="c5")
        nc.sync.dma_start(out=ghi_t, in_=ghi_e[:, :])
        gw_t = consts.tile([58, W], BF16, tag="c6")
        nc.sync.dma_start(out=gw_t, in_=gw_e[:, :])
        m1_t = consts.tile([128, NB, 3, 3, 128], BF16, tag="c7")
        nc.sync.dma_start(out=m1_t, in_=m1_e[:, :, :, :, :].transpose((3, 0, 1, 2, 4)))
        m2_t = consts.tile([128, NB, 3, 3, 128], BF16, tag="c8")
        nc.sync.dma_start(out=m2_t, in_=m2_e[:, :, :, :, :].transpose((3, 0, 1, 2, 4)))
        b1p_t = consts.tile([128, NB * 3], F32, tag="c9")
        nc.sync.dma_start(out=b1p_t, in_=b1p_e[:, :].transpose((1, 0)))
        b2e1_t = consts.tile([128, NB * 3], F32, tag="ca")
        nc.sync.dma_start(out=b2e1_t, in_=b2e1_e[:, :].transpose((1, 0)))
        b2e2_t = consts.tile([128, NB * 3], F32, tag="cb")
        nc.sync.dma_start(out=b2e2_t, in_=b2e2_e[:, :].transpose((1, 0)))
        ident = consts.tile([128, 128], BF16, tag="cid")
        make_identity(nc, ident[:, :])

        def evict(dst, src, bias=None):
            # weighted round-robin DVE:ACT = 3:2 (DVE per-elem is cheaper)
            r = ev[0] % 5
            ev[0] += 1
            if bias is None:
                if r < 3:
                    nc.vector.tensor_copy(dst, src)
                else:
                    nc.scalar.activation(dst, src, AF.Copy)
            else:
                # relu(src + bias)
                if r < 3:
                    nc.vector.tensor_scalar(dst, src, bias, 0.0, ALU.add, ALU.max)
                else:
                    nc.scalar.activation(dst, src, AF.Relu, bias=bias, scale=1.0)

        NHC = H * BS            # 10752
        HH = H // 2             # 28
        NHC2 = HH * BS          # 5376 = 12*448
        NWC = WF * BS           # 5568

        for b in range(n_b):
            for cq in range(n_cq):
                cs = slice(cq * BS, (cq + 1) * BS)

                # ---- S1: load x[b] c-slice as (w | h, c), cast f32->bf16
                xw = big.tile([W, H, BS], BF16, tag="xw")
                nc.gpsimd.dma_start(
                    out=xw, in_=x_ext[b, :, :, cs].transpose((1, 0, 2)))
                xw_f = xw[:, :, :].rearrange("w h c -> w (h c)")

                # ---- S2: Wfft -> YW_fold ((fh, r, w') | h/2, c)
                # fold pairs chunk j (h first-half) with chunk j (h second
                # half) stacked on partitions 0:58 / 58:116, single evict.
                ywf = big.tile([122, HH, BS], BF16, tag="yw")
                ywf_f = ywf[:, :, :].rearrange("p h c -> p (h c)")
                for j in range(12):
                    sl = slice(448 * j, 448 * (j + 1))
                    pw = ps.tile([122, 448], F32, tag="ps", name=f"s2_{b}_{cq}_{j}")
                    nc.tensor.matmul(pw[0:58, :], fw_t[:, :],
                                     xw_f[:, 448 * j:448 * (j + 1)],
                                     start=True, stop=True)
                    nc.tensor.matmul(pw[64:122, :], fw_t[:, :],
                                     xw_f[:, NHC2 + 448 * j:NHC2 + 448 * (j + 1)],
                                     start=True, stop=True)
                    evict(ywf_f[:, sl], pw[:, :])

                # ---- S3: rot1 -> YH ((r,h) | w', c), via DRAM bounce
                s1 = dram.tile([58, H, BS], BF16, tag="s1")
                for f in range(2):
                    nc.sync.dma_start(out=s1[:, 28 * f:28 * f + 28, :],
                                      in_=ywf[64 * f:64 * f + 58, :, :])
                yh = mid.tile([112, WF, BS], BF16, tag="yh")
                for r in range(2):
                    nc.sync.dma_start(
                        out=yh[56 * r:56 * r + 56, :, :],
                        in_=s1[29 * r:29 * r + 29, :, :].transpose((1, 0, 2)))
                yh_f = yh[:, :, :].rearrange("p w c -> p (w c)")

                # ---- S4: Hfft -> XF (h' | w', c, r)
                xf = big.tile([H, WF, BS, 2], BF16, tag="xf")
                xf_r = xf[:, :, :, 0].rearrange("q w c -> q (w c)")
                xf_i = xf[:, :, :, 1].rearrange("q w c -> q (w c)")
                for j in range(11):
                    lo = 512 * j
                    hi = min(512 * (j + 1), NWC)
                    n = hi - lo
                    sl = slice(lo, hi)
                    pxr = ps.tile([56, 512], F32, tag="ps", name=f"s4a_{b}_{cq}_{j}")
                    pxi = ps.tile([56, 512], F32, tag="ps", name=f"s4b_{b}_{cq}_{j}")
                    nc.tensor.matmul(pxr[:, :n], fhr_t[:, :], yh_f[:, sl],
                                     start=True, stop=True)
                    nc.tensor.matmul(pxi[:, :n], fhi_t[:, :], yh_f[:, sl],
                                     start=True, stop=True)
                    evict(xf_r[:, sl], pxr[:, :n])
                    evict(xf_i[:, sl], pxi[:, :n])

                # ---- S5: rot2 -> XM[kc] ((2c+r) | h', w') via PE transpose
                # two w' per PSUM tile (free offsets), single evict
                xm = []
                for kc in range(3):
                    t = xmp.tile([128, H, WF], BF16, tag="xm", name=f"xm{b}_{cq}_{kc}")
                    for wp in range(0, WF, 2):
                        wn = min(2, WF - wp)
                        pt = pst.tile([128, 112], BF16, tag="pst",
                                      name=f"s5_{b}_{cq}_{kc}_{wp}")
                        for k in range(wn):
                            nc.tensor.transpose(
                                pt[:, 56 * k:56 * k + 56],
                                xf[:, wp + k, 64 * kc:64 * kc + 64, :].rearrange(
                                    "q c r -> q (c r)"),
                                ident[:H, :H])
                        evict(t[:, :, wp:wp + wn].transpose((0, 2, 1)),
                              pt[:, :56 * wn])
                    xm.append(t)
                xm_f = [t[:, :, :].rearrange("p h w -> p (h w)") for t in xm]

                # ---- S6: M1 (relu(. + b1)) -> HM
                hm = [hmp.tile([128, H, WF], BF16, tag="hm", name=f"hm{b}_{cq}_{i}") for i in range(3)]
                hm_f = [t[:, :, :].rearrange("p h w -> p (h w)") for t in hm]
                for mc in range(3):
                    bidx = cq * 3 + mc
                    for j in range(4):
                        sl = slice(406 * j, 406 * (j + 1))
                        pm = ps.tile([128, 406], F32, tag="ps", name=f"s6_{b}_{cq}_{mc}_{j}")
                        for kc in range(3):
                            nc.tensor.matmul(
                                pm[:, :], m1_t[:, cq, kc, mc, :], xm_f[kc][:, sl],
                                start=(kc == 0), stop=(kc == 2))
                        evict(hm_f[mc][:, sl], pm[:, :],
                              bias=b1p_t[:, bidx:bidx + 1])

                # ---- S7: M2 (softshrink(. + b2)) -> OM
                # u = v + b2 (ACT); c = clamp(u, -lam, lam) (DVE);
                # om = u - c (GPSIMD, bf16)
                om = [omp.tile([128, H, WF], BF16, tag="om", name=f"om{b}_{cq}_{i}") for i in range(3)]
                om_f = [t[:, :, :].rearrange("p h w -> p (h w)") for t in om]
                for mc in range(3):
                    bidx = cq * 3 + mc
                    for j in range(4):
                        sl = slice(406 * j, 406 * (j + 1))
                        pm = ps.tile([128, 406], F32, tag="ps", name=f"s7_{b}_{cq}_{mc}_{j}")
                        for kc in range(3):
                            nc.tensor.matmul(
                                pm[:, :], m2_t[:, cq, kc, mc, :], hm_f[kc][:, sl],
                                start=(kc == 0), stop=(kc == 2))
                        tu = tmp.tile([128, 406], BF16, tag="tu",
                                      name=f"tu_{b}_{cq}_{mc}_{j}")
                        tc_ = tmp.tile([128, 406], BF16, tag="tc",
                                       name=f"tc_{b}_{cq}_{mc}_{j}")
                        nc.scalar.activation(
                            tu[:, :], pm[:, :], AF.Relu,
                            bias=b2e1_t[:, bidx:bidx + 1], scale=1.0)
                        nc.scalar.activation(
                            tc_[:, :], pm[:, :], AF.Relu,
                            bias=b2e2_t[:, bidx:bidx + 1], scale=-1.0)
                        nc.gpsimd.tensor_tensor(
                            om_f[mc][:, sl], tu[:, :], tc_[:, :], ALU.subtract)

                # ---- S8: rot3 -> OC ((wpar, h') | w'/2, c, r) via PE transpose
                # pairs (wp, wp+1) stacked on partitions 0:56 / 56:112
                oc = big.tile([120, 15, BS, 2], BF16, tag="oc")
                for mc in range(3):
                    for wp in range(0, WF, 2):
                        wn = min(2, WF - wp)
                        np_ = 56 if wn == 1 else 120
                        pt2 = pst.tile([120, 128], BF16, tag="pst",
                                       name=f"s8_{b}_{cq}_{mc}_{wp}")
                        for k in range(wn):
                            nc.tensor.transpose(
                                pt2[64 * k:64 * k + 56, :],
                                om[mc][:, :, wp + k], ident[:, :])
                        evict(oc[:np_, wp // 2, 64 * mc:64 * mc + 64, :]
                              .rearrange("q c r -> q (c r)"), pt2[:np_, :])

                # ---- S9: iH -> VH ((r,h) | wpar, w'/2, c)
                # per w'-parity: contiguous N over (w'/2, c)
                vh = mid.tile([112, 2, 15, BS], BF16, tag="vh")
                for par in range(2):
                    nw = 15 if par == 0 else 14
                    nn = nw * BS           # 2880 or 2688
                    cw = 480 if par == 0 else 448
                    oc_r = oc[64 * par:64 * par + 56, :, :, 0].rearrange(
                        "q w c -> q (w c)")
                    oc_i = oc[64 * par:64 * par + 56, :, :, 1].rearrange(
                        "q w c -> q (w c)")
                    vh_f = vh[:, par, :, :].rearrange("p w c -> p (w c)")
                    for j in range(6):
                        sl = slice(cw * j, cw * (j + 1))
                        gsl = slice(64 * par, 64 * par + 56)
                        pv = ps.tile([112, 480], F32, tag="ps", name=f"s9_{b}_{cq}_{par}_{j}")
                        nc.tensor.matmul(pv[:, :cw], ghr_t[gsl, :], oc_r[:, sl],
                                         start=True, stop=False)
                        nc.tensor.matmul(pv[:, :cw], ghi_t[gsl, :], oc_i[:, sl],
                                         start=False, stop=True)
                        evict(vh_f[:, sl], pv[:, :cw])

                # ---- S10: rot4 -> VW ((r,w') | h, c), via DRAM bounce
                s4 = dram.tile([112, 2, 15, BS], BF16, tag="s4")
                nc.sync.dma_start(out=s4[:, :, :, :], in_=vh[:, :, :, :])
                vw = big.tile([58, H, BS], BF16, tag="yw2")
                for r in range(2):
                    for par in range(2):
                        nw = 15 if par == 0 else 14
                        # dst rows parity-major: (r, par, wh); gw is permuted
                        # to match
                        o = 29 * r + 15 * par
                        nc.sync.dma_start(
                            out=vw[o:o + nw, :, :],
                            in_=s4[56 * r:56 * r + 56, par, :nw, :].transpose(
                                (1, 0, 2)))
                vw_f = vw[:, :, :].rearrange("p h c -> p (h c)")

                # ---- S11: iW -> OUT_fold ((fh, w) | h/2, c)
                outt = big.tile([120, HH, BS], BF16, tag="xw2")
                outt_f = outt[:, :, :].rearrange("w h c -> w (h c)")
                for j in range(12):
                    sl = slice(448 * j, 448 * (j + 1))
                    po = ps.tile([120, 448], F32, tag="ps", name=f"s11_{b}_{cq}_{j}")
                    nc.tensor.matmul(po[0:56, :], gw_t[:, :],
                                     vw_f[:, 448 * j:448 * (j + 1)],
                                     start=True, stop=True)
                    nc.tensor.matmul(po[64:120, :], gw_t[:, :],
                                     vw_f[:, NHC2 + 448 * j:NHC2 + 448 * (j + 1)],
                                     start=True, stop=True)
                    evict(outt_f[:, sl], po[:, :])

                # ---- S12: store (cast bf16->f32), one DMA per h-half
                for f in range(2):
                    nc.gpsimd.dma_start(
                        out=out_ext[b, 28 * f:28 * f + 28, :, cs].transpose(
                            (1, 0, 2)),
                        in_=outt[64 * f:64 * f + 56, :, :])

    nc.compile()
    return nc


_NC_CACHE = {}


def _get_nc(n_b, n_cq):
    key = (n_b, n_cq)
    if key not in _NC_CACHE:
        _NC_CACHE[key] = build_nc(n_b, n_cq)
    return _NC_CACHE[key]


def build_in_maps(inputs):
    x = np.ascontiguousarray(np.asarray(inputs["x"], dtype=np.float32))
    consts = make_consts(np.asarray(inputs["w1"]), np.asarray(inputs["b1"]),
                         np.asarray(inputs["w2"]), np.asarray(inputs["b2"]))
    in_maps = []
    for core in range(NCORES):
        shard = np.ascontiguousarray(
            x[core * BPC:(core + 1) * BPC].reshape(BPC, H, W, C))
        m = {"x": shard}
        m.update(consts)
        in_maps.append(m)
    return in_maps


def kernel(x, w1, b1, w2, b2):
    in_maps = build_in_maps(dict(x=x, w1=w1, b1=b1, w2=w2, b2=b2))
    nc = _get_nc(BPC, NB)
    res = run_bass_kernel_spmd(nc, in_maps, core_ids=list(range(NCORES)))
    out = np.concatenate(
        [res.results[i]["out"].reshape(BPC, H * W, C) for i in range(NCORES)],
        axis=0)
    return out.astype(np.float32)


# revision 16
# speedup vs baseline: 2.4264x; 1.7527x over previous
"""AFNO (Adaptive Fourier Neural Operator) Trainium2 kernel.

Data-parallel over batch: 32 batches -> 8 cores x 4 batches.
Per core, per (batch, c-quarter=block) the pipeline is 6 matmul phases with
DMA layout rotations between them (all compute bf16, fp32 PSUM):

  x (w | h,c)            --Wfft-->   YW ((fh,r,w') | h/2,c)   [rfft over w]
  rot1 (DMA)             -->         YH ((r,h)  | w',c)
  Hfft (2 stationaries)  -->         XF (h' | w',c,r)       [fft over h]
  rot2 (PE transpose)    -->         XM ((2c+r) | h',w') x3  [c to partitions]
  M1 (complex-packed)    -->         HM  (relu(.+b1))
  M2                     -->         OM  (softshrink(.+b2))
  rot3 (PE transpose)    -->         OC ((wpar,h') | w'/2,c,r)
  iH (2-pass accum)      -->         VH ((r,h) | wpar,w'/2,c)
  rot4 (DMA)             -->         VW ((r,w') | h,c)
  iW                     -->         OUT ((fh,w) | h/2,c) -> DRAM

Throughput notes vs the naive version: PSUM evictions are partition-packed
(two 56/58-row matmul results stacked into one 112/116-row PSUM tile and
evicted once - DVE/ACT cost scales only with free-dim elements), softshrink
is computed as u - clamp(u, -lam, lam) with the subtract on the otherwise
idle GPSIMD engine, and evictions are weight-balanced across DVE and ACT.
"""
import numpy as np
import ml_dtypes
from contextlib import ExitStack

import concourse.bass as bass
import concourse.mybir as mybir
import concourse.tile as tile
from concourse import bacc
from concourse.bass_utils import run_bass_kernel_spmd
from concourse.masks import make_identity

H = 56
W = 56
WF = 29
C = 768
NB = 4
BS = 192
LAM = 0.01
NCORES = 8
B_FULL = 32
BPC = B_FULL // NCORES  # 4

F32 = mybir.dt.float32
BF16 = mybir.dt.bfloat16
AF = mybir.ActivationFunctionType
ALU = mybir.AluOpType

BF = ml_dtypes.bfloat16


def make_consts(w1, b1, w2, b2):
    """Pack DFT matrices and mixing weights/biases host-side (numpy)."""
    w = np.arange(W)
    wp = np.arange(WF)
    ang = 2 * np.pi * np.outer(wp, w) / W
    Cw = np.cos(ang) / np.sqrt(W)
    Sw = np.sin(ang) / np.sqrt(W)
    h = np.arange(H)
    angh = 2 * np.pi * np.outer(h, h) / H
    Ch = np.cos(angh) / np.sqrt(H)  # [h', h] (symmetric)
    Sh = np.sin(angh) / np.sqrt(H)
    Chi, Shi = Ch, Sh  # cos/sin(2pi h h'/H)/sqrt(H); symmetric matrices
    alpha = np.full(WF, 2.0)
    alpha[0] = 1.0
    alpha[WF - 1] = 1.0
    A = (alpha[None, :] * np.cos(2 * np.pi * np.outer(w, wp) / W)) / np.sqrt(W)
    Bm = (-alpha[None, :] * np.sin(2 * np.pi * np.outer(w, wp) / W)) / np.sqrt(W)
    Bm[:, 0] = 0.0
    Bm[:, WF - 1] = 0.0

    # Wfft stationary: (56w, 58): cols [Yr = Cw x | Yi = -Sw x]
    fw = np.zeros((W, 58), np.float32)
    fw[:, :WF] = Cw.T
    fw[:, WF:] = -Sw.T

    # Hfft stationaries, moving rows = [Yr(h) ; Yi(h)] (112):
    fhr = np.zeros((112, H), np.float32)  # -> Xr = Ch Yr + Sh Yi
    fhr[:H] = Ch.T
    fhr[H:] = Sh.T
    fhi = np.zeros((112, H), np.float32)  # -> Xi = Ch Yi - Sh Yr
    fhi[:H] = -Sh.T
    fhi[H:] = Ch.T

    # iH stationaries: moving = Or (pass r) / Oi (pass i), psum = [Vr ; Vi]
    # rows duplicated at partition offsets 0 and 64 so both w'-parity
    # slices of OC (base partitions 0 / 64) can use an aligned stationary
    ghr = np.zeros((120, 112), np.float32)
    ghr[:H, :H] = Chi.T  # Vr += Chi @ Or  -> ghr[h', m] = Chi[m, h']
    ghr[:H, H:] = Shi.T  # Vi += Shi @ Or
    ghr[64:64 + H] = ghr[:H]
    ghi = np.zeros((120, 112), np.float32)
    ghi[:H, :H] = -Shi.T  # Vr -= Shi @ Oi
    ghi[:H, H:] = Chi.T  # Vi += Chi @ Oi
    ghi[64:64 + H] = ghi[:H]

    # iW stationary: moving rows = [Vr(w') ; Vi(w')] (58) -> out[w],
    # with w' rows permuted parity-major (evens then odds) to match the
    # rot4 read layout: row (r, par, wh) <- original (r, w'=2*wh+par)
    gw0 = np.zeros((58, W), np.float32)
    gw0[:WF] = A.T
    gw0[WF:] = Bm.T
    perm = []
    for r in range(2):
        for par in range(2):
            for wh in range(15 if par == 0 else 14):
                perm.append(29 * r + 2 * wh + par)
    gw = gw0[np.array(perm)]

    # Mixing weights, complex-interleaved on both sides.
    # rows 2j+r over d=64kc+j; cols 2i+rp over k=64mc+i
    def pack_mix(wl):
        wr, wi = wl[0], wl[1]  # (NB, 192, 192)
        m = np.zeros((NB, 3, 3, 128, 128), np.float32)
        for blk in range(NB):
            for kc in range(3):
                ds = slice(64 * kc, 64 * kc + 64)
                for mc in range(3):
                    ks = slice(64 * mc, 64 * mc + 64)
                    blkr = wr[blk][ds, ks]  # (64, 64)
                    blki = wi[blk][ds, ks]
                    t = m[blk, kc, mc]
                    t[0::2, 0::2] = blkr   # r=0 -> Zr
                    t[1::2, 0::2] = -blki  # r=1 -> Zr
                    t[0::2, 1::2] = blki   # r=0 -> Zi
                    t[1::2, 1::2] = blkr   # r=1 -> Zi
        return m

    m1 = pack_mix(w1)
    m2 = pack_mix(w2)

    def pack_bias(bl, scale=1.0, off=0.0):
        # (NB*3, 128): [2i+rp] = scale*bl[rp][blk][64mc+i] + off
        out = np.zeros((NB * 3, 128), np.float32)
        for blk in range(NB):
            for mc in range(3):
                ks = slice(64 * mc, 64 * mc + 64)
                out[blk * 3 + mc, 0::2] = scale * bl[0][blk][ks] + off
                out[blk * 3 + mc, 1::2] = scale * bl[1][blk][ks] + off
        return out

    b1p = pack_bias(b1)
    b2e1 = pack_bias(b2, 1.0, -LAM)    # relu(v + b2 - lam)
    b2e2 = pack_bias(b2, -1.0, -LAM)   # relu(-v - b2 - lam)

    cb = lambda a: np.ascontiguousarray(a.astype(BF))
    cf = lambda a: np.ascontiguousarray(a.astype(np.float32))
    return {
        "fw": cb(fw), "fhr": cb(fhr), "fhi": cb(fhi),
        "ghr": cb(ghr), "ghi": cb(ghi), "gw": cb(gw),
        "m1": cb(m1), "m2": cb(m2),
        "b1p": cf(b1p), "b2e1": cf(b2e1), "b2e2": cf(b2e2),
    }


def build_nc(n_b=BPC, n_cq=NB):
    nc = bacc.Bacc(None, target_bir_lowering=False, debug=False)

    x_ext = nc.declare_dram_parameter("x", [n_b, H, W, C], BF16, isOutput=False)
    out_ext = nc.declare_dram_parameter("out", [n_b, H, W, C], BF16, isOutput=True)
    fw_e = nc.declare_dram_parameter("fw", [W, 58], BF16, isOutput=False)
    fhr_e = nc.declare_dram_parameter("fhr", [112, H], BF16, isOutput=False)
    fhi_e = nc.declare_dram_parameter("fhi", [112, H], BF16, isOutput=False)
    ghr_e = nc.declare_dram_parameter("ghr", [120, 112], BF16, isOutput=False)
    ghi_e = nc.declare_dram_parameter("ghi", [120, 112], BF16, isOutput=False)
    gw_e = nc.declare_dram_parameter("gw", [58, W], BF16, isOutput=False)
    m1_e = nc.declare_dram_parameter("m1", [NB, 3, 3, 128, 128], BF16, isOutput=False)
    m2_e = nc.declare_dram_parameter("m2", [NB, 3, 3, 128, 128], BF16, isOutput=False)
    b1p_e = nc.declare_dram_parameter("b1p", [NB * 3, 128], F32, isOutput=False)
    b2e1_e = nc.declare_dram_parameter("b2e1", [NB * 3, 128], F32, isOutput=False)
    b2e2_e = nc.declare_dram_parameter("b2e2", [NB * 3, 128], F32, isOutput=False)

    ev = [0]  # eviction engine round-robin counter

    with tile.TileContext(nc) as tc, ExitStack() as ctx:
        consts = ctx.enter_context(tc.tile_pool(name="consts", bufs=1))
        big = ctx.enter_context(tc.tile_pool(name="big", bufs=1))
        mid = ctx.enter_context(tc.tile_pool(name="mid", bufs=2))
        xmp = ctx.enter_context(tc.tile_pool(name="xmp", bufs=4))
        hmp = ctx.enter_context(tc.tile_pool(name="hmp", bufs=4))
        omp = ctx.enter_context(tc.tile_pool(name="omp", bufs=4))
        tmp = ctx.enter_context(tc.tile_pool(name="tmp", bufs=4))
        ps = ctx.enter_context(tc.tile_pool(name="ps", bufs=4, space="PSUM"))
        pst = ctx.enter_context(tc.tile_pool(name="pst", bufs=4, space="PSUM"))
        dram = ctx.enter_context(tc.tile_pool(name="dram", bufs=3, space="DRAM"))

        # ---- load constants ----
        fw_t = consts.tile([W, 58], BF16, tag="c1")
        nc.sync.dma_start(out=fw_t, in_=fw_e[:, :])
        fhr_t = consts.tile([112, H], BF16, tag="c2")
        nc.sync.dma_start(out=fhr_t, in_=fhr_e[:, :])
        fhi_t = consts.tile([112, H], BF16, tag="c3")
        nc.sync.dma_start(out=fhi_t, in_=fhi_e[:, :])
        ghr_t = consts.tile([120, 112], BF16, tag="c4")
        nc.sync.dma_start(out=ghr_t, in_=ghr_e[:, :])
        ghi_t = consts.tile([120, 112], BF16, tag="c5")
        nc.sync.dma_start(out=ghi_t, in_=ghi_e[:, :])
        gw_t = consts.tile([58, W], BF16, tag="c6")
        nc.sync.dma_start(out=gw_t, in_=gw_e[:, :])
        m1_t = consts.tile([128, NB, 3, 3, 128], BF16, tag="c7")
        nc.sync.dma_start(out=m1_t, in_=m1_e[:, :, :, :, :].transpose((3, 0, 1, 2, 4)))
        m2_t = consts.tile([128, NB, 3, 3, 128], BF16, tag="c8")
        nc.sync.dma_start(out=m2_t, in_=m2_e[:, :, :, :, :].transpose((3, 0, 1, 2, 4)))
        b1p_t = consts.tile([128, NB * 3], F32, tag="c9")
        nc.sync.dma_start(out=b1p_t, in_=b1p_e[:, :].transpose((1, 0)))
        b2e1_t = consts.tile([128, NB * 3], F32, tag="ca")
        nc.sync.dma_start(out=b2e1_t, in_=b2e1_e[:, :].transpose((1, 0)))
        b2e2_t = consts.tile([128, NB * 3], F32, tag="cb")
        nc.sync.dma_start(out=b2e2_t, in_=b2e2_e[:, :].transpose((1, 0)))
        ident = consts.tile([128, 128], BF16, tag="cid")
        make_identity(nc, ident[:, :])

        def evict(dst, src, bias=None):
            # weighted round-robin DVE:ACT = 3:2 (DVE per-elem is cheaper)
            r = ev[0] % 5
            ev[0] += 1
            if bias is None:
                if r < 3:
                    nc.vector.tensor_copy(dst, src)
                else:
                    nc.scalar.activation(dst, src, AF.Copy)
            else:
                # relu(src + bias)
                if r < 3:
                    nc.vector.tensor_scalar(dst, src, bias, 0.0, ALU.add, ALU.max)
                else:
                    nc.scalar.activation(dst, src, AF.Relu, bias=bias, scale=1.0)

        NHC = H * BS            # 10752
        HH = H // 2             # 28
        NHC2 = HH * BS          # 5376 = 12*448
        NWC = WF * BS           # 5568

        for b in range(n_b):
            for cq in range(n_cq):
                cs = slice(cq * BS, (cq + 1) * BS)

                # ---- S1: load x[b] c-slice as (w | h, c), cast f32->bf16
                xw = big.tile([W, H, BS], BF16, tag="xw")
                nc.gpsimd.dma_start(
                    out=xw, in_=x_ext[b, :, :, cs].transpose((1, 0, 2)))

                xw_f = xw[:, :, :].rearrange("w h c -> w (h c)")

                # ---- S2: Wfft -> YW_fold ((fh, r, w') | h/2, c)
                # fold pairs chunk j (h first-half) with chunk j (h second
                # half) stacked on partitions 0:58 / 58:116, single evict.
                ywf = big.tile([122, HH, BS], BF16, tag="yw")
                ywf_f = ywf[:, :, :].rearrange("p h c -> p (h c)")
                for j in range(12):
                    sl = slice(448 * j, 448 * (j + 1))
                    pw = ps.tile([122, 448], F32, tag="ps", name=f"s2_{b}_{cq}_{j}")
                    nc.tensor.matmul(pw[0:58, :], fw_t[:, :],
                                     xw_f[:, 448 * j:448 * (j + 1)],
                                     start=True, stop=True)
                    nc.tensor.matmul(pw[64:122, :], fw_t[:, :],
                                     xw_f[:, NHC2 + 448 * j:NHC2 + 448 * (j + 1)],
                                     start=True, stop=True)
                    evict(ywf_f[:, sl], pw[:, :])

                # ---- S3: rot1 -> YH ((r,h) | w', c), via DRAM bounce
                s1 = dram.tile([58, H, BS], BF16, tag="s1")
                for f, eng in ((0, nc.sync), (1, nc.scalar)):
                    eng.dma_start(out=s1[:, 28 * f:28 * f + 28, :],
                                  in_=ywf[64 * f:64 * f + 58, :, :])
                yh = mid.tile([112, WF, BS], BF16, tag="yh")
                for r, eng in ((0, nc.sync), (1, nc.scalar)):
                    eng.dma_start(
                        out=yh[56 * r:56 * r + 56, :, :],
                        in_=s1[29 * r:29 * r + 29, :, :].transpose((1, 0, 2)))
                yh_f = yh[:, :, :].rearrange("p w c -> p (w c)")

                # ---- S4: Hfft -> XF (h' | w', c, r)
                xf = big.tile([H, WF, BS, 2], BF16, tag="xf")
                xf_r = xf[:, :, :, 0].rearrange("q w c -> q (w c)")
                xf_i = xf[:, :, :, 1].rearrange("q w c -> q (w c)")
                for j in range(11):
                    lo = 512 * j
                    hi = min(512 * (j + 1), NWC)
                    n = hi - lo
                    sl = slice(lo, hi)
                    pxr = ps.tile([56, 512], F32, tag="ps", name=f"s4a_{b}_{cq}_{j}")
                    pxi = ps.tile([56, 512], F32, tag="ps", name=f"s4b_{b}_{cq}_{j}")
                    nc.tensor.matmul(pxr[:, :n], fhr_t[:, :], yh_f[:, sl],
                                     start=True, stop=True)
                    nc.tensor.matmul(pxi[:, :n], fhi_t[:, :], yh_f[:, sl],
                                     start=True, stop=True)
                    evict(xf_r[:, sl], pxr[:, :n])
                    evict(xf_i[:, sl], pxi[:, :n])

                # ---- S5: rot2 -> XM[kc] ((2c+r) | h', w') via PE transpose
                # two w' per PSUM tile (free offsets), single evict
                xm = []
                for kc in range(3):
                    t = xmp.tile([128, H, WF], BF16, tag="xm", name=f"xm{b}_{cq}_{kc}")
                    for wp in range(0, WF, 2):
                        wn = min(2, WF - wp)
                        pt = pst.tile([128, 112], BF16, tag="pst",
                                      name=f"s5_{b}_{cq}_{kc}_{wp}")
                        for k in range(wn):
                            nc.tensor.transpose(
                                pt[:, 56 * k:56 * k + 56],
                                xf[:, wp + k, 64 * kc:64 * kc + 64, :].rearrange(
                                    "q c r -> q (c r)"),
                                ident[:H, :H])
                        evict(t[:, :, wp:wp + wn].transpose((0, 2, 1)),
                              pt[:, :56 * wn])
                    xm.append(t)
                xm_f = [t[:, :, :].rearrange("p h w -> p (h w)") for t in xm]

                # ---- S6: M1 (relu(. + b1)) -> HM
                hm = [hmp.tile([128, H, WF], BF16, tag="hm", name=f"hm{b}_{cq}_{i}") for i in range(3)]
                hm_f = [t[:, :, :].rearrange("p h w -> p (h w)") for t in hm]
                for mc in range(3):
                    bidx = cq * 3 + mc
                    for j in range(4):
                        sl = slice(406 * j, 406 * (j + 1))
                        pm = ps.tile([128, 406], F32, tag="ps", name=f"s6_{b}_{cq}_{mc}_{j}")
                        for kc in range(3):
                            nc.tensor.matmul(
                                pm[:, :], m1_t[:, cq, kc, mc, :], xm_f[kc][:, sl],
                                start=(kc == 0), stop=(kc == 2))
                        evict(hm_f[mc][:, sl], pm[:, :],
                              bias=b1p_t[:, bidx:bidx + 1])

                # ---- S7: M2 (softshrink(. + b2)) -> OM
                # u = v + b2 (ACT); c = clamp(u, -lam, lam) (DVE);
                # om = u - c (GPSIMD, bf16)
                om = [omp.tile([128, H, WF], BF16, tag="om", name=f"om{b}_{cq}_{i}") for i in range(3)]
                om_f = [t[:, :, :].rearrange("p h w -> p (h w)") for t in om]
                for mc in range(3):
                    bidx = cq * 3 + mc
                    for j in range(4):
                        sl = slice(406 * j, 406 * (j + 1))
                        pm = ps.tile([128, 406], F32, tag="ps", name=f"s7_{b}_{cq}_{mc}_{j}")
                        for kc in range(3):
                            nc.tensor.matmul(
                                pm[:, :], m2_t[:, cq, kc, mc, :], hm_f[kc][:, sl],
                                start=(kc == 0), stop=(kc == 2))
                        tu = tmp.tile([128, 406], BF16, tag="tu",
                                      name=f"tu_{b}_{cq}_{mc}_{j}")
                        tc_ = tmp.tile([128, 406], BF16, tag="tc",
                                       name=f"tc_{b}_{cq}_{mc}_{j}")
                        nc.scalar.activation(
                            tu[:, :], pm[:, :], AF.Relu,
                            bias=b2e1_t[:, bidx:bidx + 1], scale=1.0)
                        nc.scalar.activation(
                            tc_[:, :], pm[:, :], AF.Relu,
                            bias=b2e2_t[:, bidx:bidx + 1], scale=-1.0)
                        nc.gpsimd.tensor_tensor(
                            om_f[mc][:, sl], tu[:, :], tc_[:, :], ALU.subtract)

                # ---- S8: rot3 -> OC ((wpar, h') | w'/2, c, r) via PE transpose
                # pairs (wp, wp+1) stacked on partitions 0:56 / 56:112
                oc = big.tile([120, 15, BS, 2], BF16, tag="oc")
                for mc in range(3):
                    for wp in range(0, WF, 2):
                        wn = min(2, WF - wp)
                        np_ = 56 if wn == 1 else 120
                        pt2 = pst.tile([120, 128], BF16, tag="pst",
                                       name=f"s8_{b}_{cq}_{mc}_{wp}")
                        for k in range(wn):
                            nc.tensor.transpose(
                                pt2[64 * k:64 * k + 56, :],
                                om[mc][:, :, wp + k], ident[:, :])
                        evict(oc[:np_, wp // 2, 64 * mc:64 * mc + 64, :]
                              .rearrange("q c r -> q (c r)"), pt2[:np_, :])

                # ---- S9: iH -> VH ((r,h) | wpar, w'/2, c)
                # per w'-parity: contiguous N over (w'/2, c)
                vh = mid.tile([112, 2, 15, BS], BF16, tag="vh")
                for par in range(2):
                    nw = 15 if par == 0 else 14
                    nn = nw * BS           # 2880 or 2688
                    cw = 480 if par == 0 else 448
                    oc_r = oc[64 * par:64 * par + 56, :, :, 0].rearrange(
                        "q w c -> q (w c)")
                    oc_i = oc[64 * par:64 * par + 56, :, :, 1].rearrange(
                        "q w c -> q (w c)")
                    vh_f = vh[:, par, :, :].rearrange("p w c -> p (w c)")
                    for j in range(6):
                        sl = slice(cw * j, cw * (j + 1))
                        gsl = slice(64 * par, 64 * par + 56)
                        pv = ps.tile([112, 480], F32, tag="ps", name=f"s9_{b}_{cq}_{par}_{j}")
                        nc.tensor.matmul(pv[:, :cw], ghr_t[gsl, :], oc_r[:, sl],
                                         start=True, stop=False)
                        nc.tensor.matmul(pv[:, :cw], ghi_t[gsl, :], oc_i[:, sl],
                                         start=False, stop=True)
                        evict(vh_f[:, sl], pv[:, :cw])

                # ---- S10: rot4 -> VW ((r,w') | h, c), via DRAM bounce
                s4 = dram.tile([112, 2, 15, BS], BF16, tag="s4")
                for par, eng in ((0, nc.sync), (1, nc.scalar)):
                    eng.dma_start(out=s4[:, par, :, :], in_=vh[:, par, :, :])
                vw = big.tile([58, H, BS], BF16, tag="yw2")
                for r in range(2):
                    for par in range(2):
                        nw = 15 if par == 0 else 14
                        # dst rows parity-major: (r, par, wh); gw is permuted
                        # to match
                        o = 29 * r + 15 * par
                        eng = nc.sync if (r + par) % 2 == 0 else nc.scalar
                        eng.dma_start(
                            out=vw[o:o + nw, :, :],
                            in_=s4[56 * r:56 * r + 56, par, :nw, :].transpose(
                                (1, 0, 2)))
                vw_f = vw[:, :, :].rearrange("p h c -> p (h c)")

                # ---- S11: iW -> OUT_fold ((fh, w) | h/2, c)
                outt = big.tile([120, HH, BS], BF16, tag="xw2")
                outt_f = outt[:, :, :].rearrange("w h c -> w (h c)")
                for j in range(12):
                    sl = slice(448 * j, 448 * (j + 1))
                    po = ps.tile([120, 448], F32, tag="ps", name=f"s11_{b}_{cq}_{j}")
                    nc.tensor.matmul(po[0:56, :], gw_t[:, :],
                                     vw_f[:, 448 * j:448 * (j + 1)],
                                     start=True, stop=True)
                    nc.tensor.matmul(po[64:120, :], gw_t[:, :],
                                     vw_f[:, NHC2 + 448 * j:NHC2 + 448 * (j + 1)],
                                     start=True, stop=True)
                    evict(outt_f[:, sl], po[:, :])

                # ---- S12: store (cast bf16->f32), one DMA per h-half
                for f in range(2):
                    nc.gpsimd.dma_start(
                        out=out_ext[b, 28 * f:28 * f + 28, :, cs].transpose(
                            (1, 0, 2)),
                        in_=outt[64 * f:64 * f + 56, :, :])

    nc.compile()
    return nc


_NC_CACHE = {}


def _get_nc(n_b, n_cq):
    key = (n_b, n_cq)
    if key not in _NC_CACHE:
        _NC_CACHE[key] = build_nc(n_b, n_cq)
    return _NC_CACHE[key]


def build_in_maps(inputs):
    x = np.ascontiguousarray(np.asarray(inputs["x"], dtype=np.float32))
    x = x.astype(BF)
    consts = make_consts(np.asarray(inputs["w1"]), np.asarray(inputs["b1"]),
                         np.asarray(inputs["w2"]), np.asarray(inputs["b2"]))
    in_maps = []
    for core in range(NCORES):
        shard = np.ascontiguousarray(
            x[core * BPC:(core + 1) * BPC].reshape(BPC, H, W, C))
        m = {"x": shard}
        m.update(consts)
        in_maps.append(m)
    return in_maps


def kernel(x, w1, b1, w2, b2):
    in_maps = build_in_maps(dict(x=x, w1=w1, b1=b1, w2=w2, b2=b2))
    nc = _get_nc(BPC, NB)
    res = run_bass_kernel_spmd(nc, in_maps, core_ids=list(range(NCORES)))
    out = np.concatenate(
        [res.results[i]["out"].reshape(BPC, H * W, C) for i in range(NCORES)],
        axis=0)
    return out.astype(np.float32)
